# revision 39
# baseline (speedup 1.0000x reference)
"""GQA causal attention (window == seqlen) on 8 TRN2 NeuronCores.

Sharding: tensor-parallel over heads. Each core owns 4 contiguous Q heads
(= one KV-head group), computes its slice of Q/K/V projections, RoPE,
causal attention, and its partial contribution attn_c @ wo_c to the output;
the host sums the 8 partial outputs (bf16 partials, f32 accumulate).

Schedule: the exp() of the score tiles runs only on the Scalar (ACT)
engine at ~1.25ns/col, which is ~1.75x slower than the Tensor engine's
matmul work in a naive phase-by-phase order.  So attention is software-
pipelined into the projection phases: each attention unit (h, j) has its
score tiles + exp *paced* into the fb-loop of a later QKV token-block
(or an out-projection block for j == 3), so the Scalar engine always has
PE work shadowing it.  PSUM plan (8 banks): psA 1, psB 1, st 2, av 2,
tp 2.  PSUM->SBUF copies of transposed tiles run on Scalar (Copy shares
the act table with Exp, so no table reloads); RoPE/normalize/output
copies run on Vector.
"""

import numpy as np
import collections

B, S, DIM = 2, 2048, 4096
NH, NKV, HD = 32, 8, 128
SCALE = HD ** -0.5
NCORES = 8
QH = NH // NCORES          # 4 q heads per core (one kv head)
TOK = B * S                # 4096 flattened tokens
TB = TOK // 128            # 32 token blocks
SB = S // 128              # 16 token blocks per batch
FB = DIM // 128            # 32 feature blocks
NEG = -1e9

_cache = {}


def _build():
    import concourse.bass as bass
    import concourse.mybir as mybir
    import concourse.tile as tile
    from concourse import bacc
    from concourse.masks import make_identity

    dt = mybir.dt
    nc = bacc.Bacc("TRN2", target_bir_lowering=False, debug=False,
                   num_devices=NCORES)

    xT = nc.dram_tensor("xT", [TB, 128, FB * 128], dt.bfloat16,
                        kind="ExternalInput").ap()
    wqkv = nc.dram_tensor("wqkv", [128, FB * 768], dt.bfloat16,
                          kind="ExternalInput").ap()
    wo4 = nc.dram_tensor("wo4", [128, QH * DIM], dt.bfloat16,
                         kind="ExternalInput").ap()
    cs4 = nc.dram_tensor("cs4", [SB, 128, 512], dt.float32,
                         kind="ExternalInput").ap()
    diag = nc.dram_tensor("diag", [128, 128], dt.float32,
                          kind="ExternalInput").ap()
    out = nc.dram_tensor("out", [TOK, DIM], dt.bfloat16,
                         kind="ExternalOutput").ap()

    EXP = mybir.ActivationFunctionType.Exp

    with tile.TileContext(nc) as tc:
        with (
            tc.tile_pool(name="const", bufs=1) as constp,
            tc.tile_pool(name="wqkvp", bufs=1) as wqkvp,
            tc.tile_pool(name="wop", bufs=1) as wop,
            tc.tile_pool(name="xtp", bufs=4) as xtp,
            tc.tile_pool(name="csp", bufs=4) as csp,
            tc.tile_pool(name="actp", bufs=1) as actp,
            tc.tile_pool(name="ropep", bufs=2) as ropep,
            tc.tile_pool(name="pp", bufs=18) as pp,
            tc.tile_pool(name="smallp", bufs=8) as smallp,
            tc.tile_pool(name="ocp", bufs=3) as ocp,
            tc.tile_pool(name="psAp", bufs=1, space="PSUM") as psAp,
            tc.tile_pool(name="psBp", bufs=1, space="PSUM") as psBp,
            tc.tile_pool(name="stp", bufs=2, space="PSUM") as stp,
            tc.tile_pool(name="avp", bufs=2, space="PSUM") as avp,
            tc.tile_pool(name="tpp", bufs=2, space="PSUM") as tpp,
        ):
            ident = constp.tile([128, 128], dt.bfloat16, tag="ident",
                                name="ident")
            make_identity(nc, ident[:])
            dmask = constp.tile([128, 128], dt.float32, tag="dmask",
                                name="dmask")
            nc.sync.dma_start(dmask[:], diag[:])
            zbias = constp.tile([128, 1], dt.float32, tag="zbias",
                                name="zbias")
            nc.vector.memset(zbias[:], 0.0)

            # ---------- per-batch activation state ----------
            state = {}

            def new_batch():
                state["QT"] = [actp.tile([128, S], dt.bfloat16, tag=f"qt{h}",
                                         name=f"qt{h}") for h in range(QH)]
                state["KT"] = actp.tile([128, S], dt.bfloat16, tag="kt",
                                        name="kt")
                state["AT"] = [actp.tile([128, S], dt.bfloat16, tag=f"at{h}",
                                         name=f"at{h}") for h in range(QH)]
                state["V"] = [None] * SB

            # prefetch queues
            xt_q = {}

            def issue_loads(b, sb):
                tb = b * SB + sb
                xt = xtp.tile([128, FB, 128], dt.bfloat16, tag="xt",
                              name="xt")
                nc.sync.dma_start(xt[:].rearrange("f fb t -> f (fb t)"),
                                  xT[tb])
                cs = csp.tile([128, 512], dt.float32, tag="cs", name="cs")
                nc.sync.dma_start(cs[:], cs4[sb])
                xt_q[(b, sb)] = (xt, cs)

            # paced work queue: closures emitting one small PE+{Vector,Scalar}
            # step each (score tiles, deferred AV transposes)
            work_q = collections.deque()

            def pace():
                if work_q:
                    work_q.popleft()()

            pt_for = {}

            def enqueue_scores(h, j):
                KT = state["KT"]
                QT = state["QT"]
                nt = 4 * j + 4
                plist = []
                pt_for[(h, j)] = plist
                # h > 0 units' QT/KT inputs were emitted >= 1 slot ago, so
                # their score tiles may jump ahead of pending transpose
                # closures whose RoPE inputs are still in the Vector queue
                front = h > 0

                def mk(i):
                    def go():
                        off = max(0, i - 4 * j) * 128
                        st = stp.tile([128, 512], dt.float32, tag="st",
                                      name="st")
                        nc.tensor.matmul(
                            st[:, off:512],
                            KT[:, i * 128:(i + 1) * 128],
                            QT[h][:, j * 512 + off:(j + 1) * 512],
                            start=True, stop=True)
                        if i >= 4 * j:
                            nc.vector.tensor_add(st[:, off:off + 128],
                                                 st[:, off:off + 128],
                                                 dmask[:])
                        pt = pp.tile([128, 512], dt.bfloat16, tag="p",
                                     name="p")
                        nc.scalar.activation(pt[:, off:512], st[:, off:512],
                                             EXP, bias=zbias[:], scale=SCALE)
                        plist.append(pt)
                    return go

                if front:
                    work_q.extendleft(mk(i) for i in reversed(range(nt)))
                else:
                    work_q.extend(mk(i) for i in range(nt))

            # ---------- QKV projection unit for one token block ----------
            def qkv_tail(sb, psA, psB, cs):
                """RoPE + V extract + transposes + copies for one token
                block whose projections are in psA (q, 512) / psB (k|v,
                256).  The PSUM banks are freed by one quick copy each so
                the next slot's accumulation can start while RoPE runs."""
                V = state["V"]
                QT = state["QT"]
                KT = state["KT"]
                cst = cs[:, 0:256]
                snt = cs[:, 256:512]
                rqf = ropep.tile([128, 512], dt.float32, tag="rqf",
                                 name="rqf", bufs=1)
                nc.vector.tensor_copy(rqf[:], psA)
                kvf = ropep.tile([128, 256], dt.float32, tag="kvf",
                                 name="kvf", bufs=1)
                nc.vector.tensor_copy(kvf[:], psB)
                # RoPE on Q: [tok, 512] interleaved pairs
                rq = ropep.tile([128, 512], dt.bfloat16, tag="rq", name="rq")
                qa = rqf[:].rearrange("p (i two) -> p two i", two=2)
                ra = rq[:].rearrange("p (i two) -> p two i", two=2)
                t1 = ropep.tile([128, 256], dt.float32, tag="t1", name="t1", bufs=1)
                t2 = ropep.tile([128, 256], dt.float32, tag="t2", name="t2", bufs=1)
                t3 = ropep.tile([128, 256], dt.float32, tag="t3", name="t3", bufs=1)
                t4 = ropep.tile([128, 256], dt.float32, tag="t4", name="t4", bufs=1)
                nc.vector.tensor_mul(t1[:], qa[:, 0, :], cst[:])
                nc.vector.tensor_mul(t2[:], qa[:, 1, :], snt[:])
                nc.vector.tensor_sub(ra[:, 0, :], t1[:], t2[:])
                nc.vector.tensor_mul(t3[:], qa[:, 0, :], snt[:])
                nc.vector.tensor_mul(t4[:], qa[:, 1, :], cst[:])
                nc.vector.tensor_add(ra[:, 1, :], t3[:], t4[:])

                # RoPE on K: [tok, 128]
                rk = ropep.tile([128, 128], dt.bfloat16, tag="rk", name="rk")
                ka = kvf[:, 0:128].rearrange("p (i two) -> p two i", two=2)
                rka = rk[:].rearrange("p (i two) -> p two i", two=2)
                t5 = ropep.tile([128, 64], dt.float32, tag="t5", name="t5", bufs=1)
                t6 = ropep.tile([128, 64], dt.float32, tag="t6", name="t6", bufs=1)
                nc.vector.tensor_mul(t5[:], ka[:, 0, :], cst[:, 0:64])
                nc.vector.tensor_mul(t6[:], ka[:, 1, :], snt[:, 0:64])
                nc.vector.tensor_sub(rka[:, 0, :], t5[:], t6[:])
                t7 = ropep.tile([128, 64], dt.float32, tag="t5", name="t7", bufs=1)
                t8 = ropep.tile([128, 64], dt.float32, tag="t6", name="t8", bufs=1)
                nc.vector.tensor_mul(t7[:], ka[:, 0, :], snt[:, 0:64])
                nc.vector.tensor_mul(t8[:], ka[:, 1, :], cst[:, 0:64])
                nc.vector.tensor_add(rka[:, 1, :], t7[:], t8[:])

                # V (no rope) + ones column for the softmax denominator
                v = actp.tile([128, HD + 1], dt.bfloat16, tag=f"v{sb}",
                              name=f"v{sb}")
                V[sb] = v
                nc.vector.memset(v[:, HD:HD + 1], 1.0)
                nc.vector.tensor_copy(v[:, 0:HD], kvf[:, 128:256])

                # Transpose Q heads and K into [d, tok] layout; deferred
                # into the pace queue so they run mid-next-slot when the
                # RoPE results are long since ready (no PE stall on the
                # Vector->PE chain).  Copies on Scalar (shares Exp table).
                def tr():
                    tp = tpp.tile([128, 5, 128], dt.bfloat16, tag="tp",
                                  name="tpqk")
                    for h in range(QH):
                        nc.tensor.transpose(tp[:, h, :],
                                            rq[:, h * 128:(h + 1) * 128],
                                            ident[:])
                    nc.tensor.transpose(tp[:, 4, :], rk[:], ident[:])
                    for h in range(QH):
                        nc.scalar.copy(QT[h][:, sb * 128:(sb + 1) * 128],
                                       tp[:, h, :])
                    nc.scalar.copy(KT[:, sb * 128:(sb + 1) * 128],
                                   tp[:, 4, :])
                work_q.append(tr)

            def emit_qkv_slot(b, sb):
                xt, cs = xt_q.pop((b, sb))
                pace()
                pace()
                psA = psAp.tile([128, 512], dt.float32, tag="psA", name="psA")
                psB = psBp.tile([128, 256], dt.float32, tag="psB", name="psB")
                for fb in range(FB):
                    nc.tensor.matmul(psA[:], xt[:, fb, :],
                                     wqkv_all[:, fb, 0:512],
                                     start=(fb == 0), stop=(fb == FB - 1))
                    nc.tensor.matmul(psB[:], xt[:, fb, :],
                                     wqkv_all[:, fb, 512:768],
                                     start=(fb == 0), stop=(fb == FB - 1))
                    if fb % 2 == 1:
                        pace()
                qkv_tail(sb, psA[:], psB[:], cs)

            def emit_qkv_slot3(b):
                """First three token blocks fb-interleaved: 6 matmuls per
                weight tile keeps the PE behind the wqkv chunk DMA stream."""
                xts = [xt_q.pop((b, i)) for i in range(3)]
                psA0 = psAp.tile([128, 512], dt.float32, tag="psA",
                                 name="psA")
                psB0 = psBp.tile([128, 256], dt.float32, tag="psB",
                                 name="psB")
                psA1 = stp.tile([128, 512], dt.float32, tag="st", name="psA1")
                psB1 = stp.tile([128, 512], dt.float32, tag="st", name="psB1")
                psA2 = avp.tile([128, 512], dt.float32, tag="av", name="psA2")
                psB2 = avp.tile([128, 256], dt.float32, tag="av", name="psB2")
                pairs = ((psA0, psB0), (psA1, psB1[:, 0:256]), (psA2, psB2))
                for fb in range(FB):
                    for i, (psA, psB) in enumerate(pairs):
                        nc.tensor.matmul(psA[:], xts[i][0][:, fb, :],
                                         wqkv_all[:, fb, 0:512],
                                         start=(fb == 0), stop=(fb == FB - 1))
                        nc.tensor.matmul(psB[:], xts[i][0][:, fb, :],
                                         wqkv_all[:, fb, 512:768],
                                         start=(fb == 0), stop=(fb == FB - 1))
                for i, (psA, psB) in enumerate(pairs):
                    qkv_tail(i, psA[:], psB[:], xts[i][1])

            # ---------- attention AV unit ----------
            def emit_av(b, h, j):
                V = state["V"]
                AT = state["AT"]
                plist = pt_for.pop((h, j))
                assert len(plist) == 4 * j + 4, (h, j, len(plist))
                ans = []
                for ml in range(4):
                    m = 4 * j + ml
                    av = avp.tile([128, 136], dt.float32, tag="av",
                                  name="av")
                    for i in range(m + 1):
                        nc.tensor.matmul(av[:, 0:HD + 1],
                                         plist[i][:, ml * 128:(ml + 1) * 128],
                                         V[i][:, 0:HD + 1],
                                         start=(i == 0), stop=(i == m))
                    rec = smallp.tile([128, 1], dt.float32, tag="rec",
                                      name="rec")
                    nc.vector.reciprocal(rec[:], av[:, HD:HD + 1])
                    an = smallp.tile([128, 128], dt.bfloat16, tag="an",
                                     name="an")
                    nc.vector.tensor_scalar_mul(an[:], av[:, 0:HD], rec[:])
                    ans.append(an)
                box = {}

                def mk(ml):
                    def go():
                        if "tp" not in box:
                            box["tp"] = tpp.tile([128, 5, 128], dt.bfloat16,
                                                 tag="tp", name="tpav")
                        tp = box["tp"]
                        nc.tensor.transpose(tp[:, ml, :], ans[ml][:],
                                            ident[:])
                        nc.scalar.copy(
                            AT[h][:, (4 * j + ml) * 128:(4 * j + ml + 1) * 128],
                            tp[:, ml, :])
                    return go

                for ml in range(4):
                    work_q.append(mk(ml))

            # ---------- output projection unit for one token block ----------
            def emit_op(b, sb, paced, AT=None, scalar_copy=False):
                if AT is None:
                    AT = state["AT"]
                for half in range(2):
                    oc = ocp.tile([128, DIM // 2], dt.bfloat16, tag="oc",
                                  name="oc")
                    for c4 in range(4):
                        ch = half * 4 + c4
                        pool = psAp if ch % 2 == 0 else psBp
                        tag = "psA" if ch % 2 == 0 else "psB"
                        ps = pool.tile([128, 512], dt.float32, tag=tag,
                                       name="op")
                        for hh in range(QH):
                            nc.tensor.matmul(
                                ps[:],
                                AT[hh][:, sb * 128:(sb + 1) * 128],
                                wo_all[:, hh, ch * 512:(ch + 1) * 512],
                                start=(hh == 0), stop=(hh == QH - 1))
                        if scalar_copy:
                            nc.scalar.copy(oc[:, c4 * 512:(c4 + 1) * 512],
                                           ps[:])
                        else:
                            nc.vector.tensor_copy(
                                oc[:, c4 * 512:(c4 + 1) * 512], ps[:])
                        if paced:
                            pace()
                    nc.sync.dma_start(
                        out[b * S + sb * 128:b * S + (sb + 1) * 128,
                            half * (DIM // 2):(half + 1) * (DIM // 2)],
                        oc[:])

            def emit_op_pair(b, sbA, sbB, AT=None):
                """Two out-projection token blocks chunk-interleaved, copies
                split across Vector/Scalar: enough independent PE work
                between PSUM-ring reuses that the copy latency never stalls
                the PE (and it stays at full p-state)."""
                if AT is None:
                    AT = state["AT"]
                for half in range(2):
                    ocA = ocp.tile([128, DIM // 2], dt.bfloat16, tag="oc",
                                   name="ocA")
                    ocB = ocp.tile([128, DIM // 2], dt.bfloat16, tag="oc",
                                   name="ocB")
                    for c4 in range(4):
                        ch = half * 4 + c4
                        for sb, oc, pool, tag, veng in (
                                (sbA, ocA, psAp, "psA", True),
                                (sbB, ocB, psBp, "psB", False)):
                            ps = pool.tile([128, 512], dt.float32, tag=tag,
                                           name="op")
                            for hh in range(QH):
                                nc.tensor.matmul(
                                    ps[:],
                                    AT[hh][:, sb * 128:(sb + 1) * 128],
                                    wo_all[:, hh, ch * 512:(ch + 1) * 512],
                                    start=(hh == 0), stop=(hh == QH - 1))
                            if veng:
                                nc.vector.tensor_copy(
                                    oc[:, c4 * 512:(c4 + 1) * 512], ps[:])
                            else:
                                nc.scalar.copy(
                                    oc[:, c4 * 512:(c4 + 1) * 512], ps[:])
                    for sb, oc in ((sbA, ocA), (sbB, ocB)):
                        nc.sync.dma_start(
                            out[b * S + sb * 128:b * S + (sb + 1) * 128,
                                half * (DIM // 2):(half + 1) * (DIM // 2)],
                            oc[:])

            # ================= emission =================
            # initial loads: first x blocks, then weights
            # weights: few big partition-major DMAs (Sync dispatch is
            # ~0.6us per dma_start; 36 small loads would serialize startup),
            # interleaved with the first x blocks so the PE can start early
            wqkv_all = wqkvp.tile([128, FB, 768], dt.bfloat16, tag="wqkv",
                                  name="wqkv")
            NCH = 8
            CW = FB // NCH * 768

            def wq_chunk(c):
                nc.sync.dma_start(
                    wqkv_all[:, c * (FB // NCH):(c + 1) * (FB // NCH), :]
                    .rearrange("p a b -> p (a b)"),
                    wqkv[:, c * CW:(c + 1) * CW])

            issue_loads(0, 0)
            wq_chunk(0)
            wq_chunk(1)
            issue_loads(0, 1)
            wq_chunk(2)
            wq_chunk(3)
            issue_loads(0, 2)
            for c in range(4, NCH):
                wq_chunk(c)
            issue_loads(0, 3)
            wo_all = wop.tile([128, QH, DIM], dt.bfloat16, tag="wo",
                              name="wo")
            for c in range(2):
                nc.sync.dma_start(
                    wo_all[:, c * 2:(c + 1) * 2, :]
                    .rearrange("p a b -> p (a b)"),
                    wo4[:, c * 2 * DIM:(c + 1) * 2 * DIM])

            for b in range(B):
                prev_AT = state.get("AT")
                new_batch()
                # prologue: token blocks 0..3
                if b == 0:
                    emit_qkv_slot3(0)
                    issue_loads(0, 4)
                    issue_loads(0, 5)
                    emit_qkv_slot(0, 3)
                    issue_loads(0, 6)
                    issue_loads(0, 7)
                else:
                    # interleave with the tail of batch 0's out-projection
                    for sb in range(4):
                        emit_qkv_slot(1, sb)
                        issue_loads(1, sb + 4)
                        emit_op(0, 12 + sb, paced=True, AT=prev_AT,
                                scalar_copy=True)
                # main pipelined loop: attention (h, j<=2) units embedded in
                # the remaining QKV token blocks
                for j in range(3):
                    for h in range(QH):
                        sb = 4 * (j + 1) + h
                        enqueue_scores(h, j)
                        emit_qkv_slot(b, sb)
                        nsb = sb + 4
                        if nsb < SB:
                            issue_loads(b, nsb)
                        elif b == 0:
                            issue_loads(1, nsb - SB)
                        emit_av(b, h, j)
                # j == 3 units embedded in out-projection blocks
                op_next = 0
                for h in range(QH):
                    enqueue_scores(h, 3)
                    for _ in range(3):
                        emit_op(b, op_next, paced=True)
                        op_next += 1
                    pace()
                    emit_av(b, h, 3)
                if b == 1:
                    # drain deferred AV transposes: the trailing out-proj
                    # units read the AT blocks those closures write
                    while work_q:
                        work_q.popleft()()
                    for sb in range(12, 16):
                        emit_op(1, sb, paced=False, scalar_copy=True)

    nc.compile()
    return nc


def _prep_host(inputs):
    import ml_dtypes
    bf16 = ml_dtypes.bfloat16

    x = np.asarray(inputs["x"], np.float32)
    wq = np.asarray(inputs["wq"], np.float32)
    wk = np.asarray(inputs["wk"], np.float32)
    wv = np.asarray(inputs["wv"], np.float32)
    wo = np.asarray(inputs["wo"], np.float32)
    cos = np.asarray(inputs["freqs_cos"], np.float32)
    sin = np.asarray(inputs["freqs_sin"], np.float32)

    x2 = x.reshape(TOK, DIM)
    xT5 = np.ascontiguousarray(
        x2.reshape(TB, 128, FB, 128).transpose(0, 3, 2, 1)
        .reshape(TB, 128, FB * 128)).astype(bf16)
    cos4 = np.tile(cos, (1, QH)).reshape(SB, 128, 256).astype(np.float32)
    sin4 = np.tile(sin, (1, QH)).reshape(SB, 128, 256).astype(np.float32)
    cs4 = np.ascontiguousarray(np.concatenate([cos4, sin4], axis=2))
    k_i = np.arange(128)[:, None]
    q_i = np.arange(128)[None, :]
    dmask = np.where(k_i <= q_i, 0.0, NEG).astype(np.float32)

    in_maps = []
    for c in range(NCORES):
        wq_c = wq[:, c * QH * HD:(c + 1) * QH * HD]
        wk_c = wk[:, c * HD:(c + 1) * HD]
        wv_c = wv[:, c * HD:(c + 1) * HD]
        wqkv_c = np.ascontiguousarray(
            np.concatenate([wq_c, wk_c, wv_c], axis=1)
            .reshape(FB, 128, 768).transpose(1, 0, 2)
            .reshape(128, FB * 768)).astype(bf16)
        wo_c = np.ascontiguousarray(
            wo[c * QH * HD:(c + 1) * QH * HD, :]
            .reshape(QH, HD, DIM).transpose(1, 0, 2)
            .reshape(128, QH * DIM)).astype(bf16)
        in_maps.append({
            "xT": xT5, "wqkv": wqkv_c, "wo4": wo_c,
            "cs4": cs4, "diag": dmask,
        })
    return in_maps


def run_on_device(inputs, trace=False, tmpdir=None):
    """Compile (cached) + run; returns (full_output, BassKernelResults)."""
    import sys
    if "/opt/trn_rl_repo" not in sys.path:
        sys.path.insert(0, "/opt/trn_rl_repo")
    from concourse.bass_utils import run_bass_kernel_spmd

    if "nc" not in _cache:
        _cache["nc"] = _build()
    nc = _cache["nc"]
    in_maps = _prep_host(inputs)
    res = run_bass_kernel_spmd(nc, in_maps, core_ids=list(range(NCORES)),
                               trace=trace, tmpdir=tmpdir)
    acc = np.zeros((TOK, DIM), np.float32)
    for c in range(NCORES):
        acc += np.asarray(res.results[c]["out"], np.float32)
    return acc.reshape(B, S, DIM), res


def kernel(**inputs):
    out, _ = run_on_device(inputs, trace=False)
    return out
